# revision 1
# baseline (speedup 1.0000x reference)
"""MoE gate kernel for Trainium2 (8 NeuronCores, SPMD).

Computes, for hidden_states [4, 4096, 4096] f32 and gate_weight [8, 4096] f32:
    logits = hidden @ gate_weight.T          # [tokens, 8]
    p      = softmax(logits)                 # [tokens, 8]
    topk_w, topk_i = top_k(p, 2); topk_w /= topk_w.sum(-1, keepdims=True)

Sharding: data-parallel over tokens (B*S = 16384 -> 2048 tokens/core), gate
weight replicated.  Per core, 128-token tiles are processed two ways (mixed
to balance DMA descriptor bandwidth against PE/DVE time):

* strided path: DMA puts h_hi = h//32 on the 128 SBUF partitions (128B DRAM
  runs -> descriptor-rate-bound but no on-chip transpose); PE contracts over
  h with 32 accumulating matmuls (lhsT = X chunk [128h x 128t stride 32],
  rhs = W chunk [128h x 8e]).
* natural path: contiguous DMA ([128 tokens x 4096], 16KB descriptors at
  full HBM rate), then each 128x128 chunk is transposed on the PE (identity
  matmul, exact in fp32), copied PSUM->SBUF on DVE, and fed to the same
  gate matmul.

Top-2 + renorm uses the DVE max/max_index sort instructions; the
renormalized weights reduce to w1 = 1/(1+exp(m2-m1)), w2 = 1-w1 (the
full-softmax denominator cancels in the reference's top-k renorm).

Walrus's TPB instruction encodings carry a single sync-wait slot, so a
post-pass hoists surplus Tile-generated waits onto same-engine
EventSemaphore prefix instructions (semantics-preserving).
"""

import numpy as np

H = 4096            # hidden size
E = 8               # experts
P = 128             # SBUF partitions
F = H // P          # 32 f32 per partition per token (strided layout)
C = H // P          # 32 128-wide h-chunks (natural layout)
T_TILE = 128        # tokens per tile (PSUM partition dim)
N_CORES = 8
TOKENS_TOTAL = 4 * 4096
TOKENS_PER_CORE = TOKENS_TOTAL // N_CORES   # 2048
N_TILES = TOKENS_PER_CORE // T_TILE         # 16


def _emit_body(nc, mybir, pools, consts, x_r, x_flat, wq, iq, n_tiles,
               nat_per_group=0, pair_dma=False, g_size=8):
    xpool, natpool, xtp, cpool, spool, psum_pool, tpsum = pools
    w_sb, w_nat, ident = consts
    td = pair_dma if isinstance(pair_dma, int) and pair_dma > 1 else (2 if pair_dma else 1)
    x_r2 = (
        x_flat.rearrange("(n t) (p f) -> n p t f", t=td * T_TILE, p=P)
        if td > 1 else None
    )
    pair_tile = [None]

    group = min(g_size, n_tiles)
    assert n_tiles % group == 0
    for g in range(n_tiles // group):
        psg = psum_pool.tile([T_TILE, group, E], mybir.dt.float32)
        for il in range(group):
            i = g * group + il
            if il < nat_per_group:
                # natural path: big-descriptor DMA, then per 128x128 chunk a
                # REGULAR identity matmul as transpose (exact in fp32; unlike
                # transpose-mode it counts as PE-busy for the HAM clock gate
                # and avoids the slow transpose datapath), DVE copy
                # PSUM->SBUF, and the gate matmul.  Emission is software-
                # pipelined LA chunks deep so the PE never stalls on the
                # PE->DVE->PE round trip.
                xt_nat = natpool.tile([P, H], mybir.dt.float32)
                nc.sync.dma_start(
                    xt_nat[:], x_flat[i * T_TILE : (i + 1) * T_TILE, :]
                )
                LA = 4
                xTcs = {}
                for cc in range(C + LA):
                    if cc < C:
                        pst = tpsum.tile(
                            [P, T_TILE], mybir.dt.float32, tag="pst"
                        )
                        nc.tensor.transpose(
                            pst[:], xt_nat[:, cc * P : (cc + 1) * P], ident[:]
                        )
                        xTc = xtp.tile([P, T_TILE], mybir.dt.float32)
                        nc.vector.tensor_copy(xTc[:], pst[:])
                        xTcs[cc] = xTc
                    if cc >= LA:
                        c = cc - LA
                        nc.tensor.matmul(
                            psg[:, il], xTcs.pop(c)[:], w_nat[:, c, :],
                            start=(c == 0), stop=(c == C - 1),
                        )
            elif td > 1:
                # one DMA covers td token-tiles (1/td the per-DMA fixed cost)
                if i % td == 0:
                    xt2 = xpool.tile([P, td * T_TILE, F], mybir.dt.float32)
                    nc.sync.dma_start(xt2[:], x_r2[i // td])
                    pair_tile[0] = xt2
                half = (i % td) * T_TILE
                xt2 = pair_tile[0]
                for j in range(F):
                    nc.tensor.matmul(
                        psg[:, il],
                        xt2[:, half : half + T_TILE, j],
                        w_sb[:, :, j],
                        start=(j == 0),
                        stop=(j == F - 1),
                    )
            else:
                xt = xpool.tile([P, T_TILE, F], mybir.dt.float32)
                nc.sync.dma_start(xt[:], x_r[i])
                for j in range(F):
                    nc.tensor.matmul(
                        psg[:, il],
                        xt[:, :, j],
                        w_sb[:, :, j],
                        start=(j == 0),
                        stop=(j == F - 1),
                    )
        logits = spool.tile([T_TILE, group, E], mybir.dt.float32)
        nc.vector.tensor_copy(logits[:], psg[:])
        for il in range(group):
            i = g * group + il
            nc.vector.max(out=cpool.sorted_w[:, i], in_=logits[:, il])
            nc.vector.max_index(
                out=cpool.idx_w[:, i], in_max=cpool.sorted_w[:, i],
                in_values=logits[:, il],
            )

    sorted_w, idx_w = cpool.sorted_w, cpool.idx_w
    # Batched renormalization over all tiles: w1 = 1/(1+e^(m2-m1)),
    # w2 = e^(m2-m1)/(1+e^(m2-m1)).
    m1 = sorted_w[:, :, 0]
    m2 = sorted_w[:, :, 1]
    d = cpool.tile([P, n_tiles], mybir.dt.float32)
    nc.vector.tensor_sub(d[:], m2, m1)
    t = cpool.tile([P, n_tiles], mybir.dt.float32)
    nc.scalar.activation(t[:], d[:], mybir.ActivationFunctionType.Exp)
    denom = cpool.tile([P, n_tiles], mybir.dt.float32)
    nc.vector.tensor_scalar_add(denom[:], t[:], 1.0)
    r = cpool.tile([P, n_tiles], mybir.dt.float32)
    nc.vector.reciprocal(r[:], denom[:])

    wout = cpool.tile([P, n_tiles, 2], mybir.dt.float32)
    nc.vector.tensor_copy(wout[:, :, 0], r[:])
    nc.vector.tensor_mul(wout[:, :, 1], t[:], r[:])
    iout = cpool.tile([P, n_tiles, 2], mybir.dt.uint32)
    nc.vector.tensor_copy(iout[:], idx_w[:, :, 0:2])

    nc.gpsimd.dma_start(wq[:], wout[:])
    nc.gpsimd.dma_start(iq[:], iout[:])


def _legalize_sync_waits(nc, mybir):
    """Split surplus sync waits onto EventSemaphore prefix instructions.

    Walrus's TPB instruction structs have a single `events` wait slot and
    reject instructions with more sync waits.  The same engine sequencer
    executes an EventSemaphore (CTRL_ES) wait-only instruction in program
    order, so hoisting all but one wait onto ES prefixes is
    semantics-preserving.
    """
    limit = 1
    n = 0
    for bb in nc.main_func.blocks:
        out, changed = [], False
        for ins in bb.instructions:
            si = ins.sync_info
            if si is not None and len(si.on_wait) > limit:
                waits = list(si.on_wait)
                for w in waits[:-limit]:
                    es = mybir.InstEventSemaphore(
                        name=f"ESleg-{n}", engine=ins.engine, ins=[], outs=[],
                        sync_info=mybir.SyncInfo(on_wait=[w], on_update=[]),
                    )
                    out.append(es)
                    n += 1
                ins.sync_info = mybir.SyncInfo(
                    on_wait=waits[-limit:], on_update=list(si.on_update)
                )
                changed = True
            out.append(ins)
        if changed:
            bb.instructions = out
    return n


def build_program(tokens_per_core: int = TOKENS_PER_CORE, reps: int = 1,
                  legalize: bool = True, nat_per_group: int = 0,
                  xpool_bufs: int = 4, pair_dma: bool = False, g_size: int = 8):
    import concourse.bass as bass
    import concourse.mybir as mybir
    from concourse.masks import make_identity
    from concourse.tile import TileContext

    n_tiles = tokens_per_core // T_TILE
    nc = bass.Bass("TRN2", debug=False)
    x = nc.declare_dram_parameter(
        "x", [tokens_per_core, H], mybir.dt.float32, isOutput=False
    )
    w = nc.declare_dram_parameter("w", [E, H], mybir.dt.float32, isOutput=False)
    wq = nc.declare_dram_parameter(
        "wq", [P, n_tiles, 2], mybir.dt.float32, isOutput=True
    )
    iq = nc.declare_dram_parameter(
        "iq", [P, n_tiles, 2], mybir.dt.uint32, isOutput=True
    )

    x_flat = x[:]
    # (p, t, f): h = p*F + f; DRAM-side runs of F contiguous f32 (128B)
    x_r = x_flat.rearrange("(n t) (p f) -> n p t f", t=T_TILE, p=P)
    # (p, e, j): W^T chunk j is w_sb[:, :, j] = W[e, p*F + j]
    w_r = w[:].rearrange("e (p j) -> p e j", p=P)

    with TileContext(nc) as tc:
        with (
            tc.tile_pool(name="xpool", bufs=xpool_bufs) as xpool,
            tc.tile_pool(name="natpool", bufs=3) as natpool,
            tc.tile_pool(name="xtp", bufs=8) as xtp,
            tc.tile_pool(name="cpool", bufs=1) as cpool,
            tc.tile_pool(name="spool", bufs=4) as spool,
            tc.tile_pool(name="psum", bufs=2, space="PSUM") as psum_pool,
            tc.tile_pool(name="tpsum", bufs=6, space="PSUM") as tpsum,
        ):
            w_sb = cpool.tile([P, E, F], mybir.dt.float32)
            nc.sync.dma_start(w_sb[:], w_r)

            w_nat = ident = None
            if nat_per_group > 0:
                ident = cpool.tile([P, P], mybir.dt.float32)
                make_identity(nc, ident[:])
                # W^T with h%128c on partitions: w_nat[p, c, e] = W[e, c*128+p].
                # The DMA gather would need 4-byte descriptors, so build it
                # with one-time PE transposes of the natural W rows instead.
                w_rows = cpool.tile([E, H], mybir.dt.float32)
                nc.sync.dma_start(w_rows[:], w[:])
                w_nat = cpool.tile([P, C, E], mybir.dt.float32)
                for c in range(C):
                    psw = tpsum.tile([P, E], mybir.dt.float32, tag="pst")
                    nc.tensor.matmul(
                        psw[:], w_rows[:, c * P : (c + 1) * P],
                        ident[0:E, 0:E], start=True, stop=True,
                    )
                    nc.vector.tensor_copy(w_nat[:, c, :], psw[:])

            # per-rep output scratch lives on cpool; stash handles on the pool
            cpool.sorted_w = cpool.tile([P, n_tiles, E], mybir.dt.float32)
            cpool.idx_w = cpool.tile([P, n_tiles, E], mybir.dt.uint32)

            pools = (xpool, natpool, xtp, cpool, spool, psum_pool, tpsum)
            for _rep in range(reps):
                _emit_body(
                    nc, mybir, pools, (w_sb, w_nat, ident),
                    x_r, x_flat, wq, iq, n_tiles,
                    nat_per_group=nat_per_group, pair_dma=pair_dma,
                    g_size=g_size,
                )
    if legalize:
        _legalize_sync_waits(nc, mybir)
    return nc


def shard_inputs(hidden_states, gate_weight):
    hs = np.ascontiguousarray(np.asarray(hidden_states, dtype=np.float32)).reshape(
        TOKENS_TOTAL, H
    )
    gw = np.ascontiguousarray(np.asarray(gate_weight, dtype=np.float32))
    return [
        {"x": hs[c * TOKENS_PER_CORE : (c + 1) * TOKENS_PER_CORE], "w": gw}
        for c in range(N_CORES)
    ]


def assemble(results):
    ws, idxs = [], []
    for c in range(N_CORES):
        wq = np.asarray(results[c]["wq"]).reshape(P, N_TILES, 2)
        iq = np.asarray(results[c]["iq"]).reshape(P, N_TILES, 2)
        # token (core-local) = tile*128 + p
        ws.append(np.transpose(wq, (1, 0, 2)).reshape(TOKENS_PER_CORE, 2))
        idxs.append(np.transpose(iq, (1, 0, 2)).reshape(TOKENS_PER_CORE, 2))
    w_full = np.concatenate(ws, 0).reshape(4, 4096, 2).astype(np.float32)
    i_full = np.concatenate(idxs, 0).reshape(4, 4096, 2).astype(np.int32)
    return w_full, i_full


BEST_CONFIG = {"nat_per_group": 0, "pair_dma": 2, "xpool_bufs": 3}


def kernel(hidden_states, gate_weight):
    from concourse.bass_utils import run_bass_kernel_spmd

    nc = build_program(**BEST_CONFIG)
    in_maps = shard_inputs(hidden_states, gate_weight)
    br = run_bass_kernel_spmd(nc, in_maps, list(range(N_CORES)), trace=False)
    return assemble(br.results)



# revision 7
# speedup vs baseline: 1.4792x; 1.4792x over previous
"""MoE gate kernel for Trainium2 (8 NeuronCores, SPMD).

Computes, for hidden_states [4, 4096, 4096] f32 and gate_weight [8, 4096] f32:
    logits = hidden @ gate_weight.T          # [tokens, 8]
    p      = softmax(logits)                 # [tokens, 8]
    topk_w, topk_i = top_k(p, 2); topk_w /= topk_w.sum(-1, keepdims=True)

Sharding: data-parallel over tokens (B*S = 16384 -> 2048 tokens/core), gate
weight replicated.

Per core the kernel targets the natural-layout DMA roofline (~358 GB/s
HBM per NeuronCore):

* DMA: contiguous [128 tokens x 4096] tiles (16 KB rows, full HBM rate) --
  unlike a strided h-major load whose 128 B descriptors cap at ~140 GB/s.
* PE: per 128x128 chunk, a transpose-mode matmul produces x^T in PSUM;
  4 chunks share one PSUM bank via one start/stop accumulation group
  writing disjoint slices (start=True pending-zeros the whole 2 KB zero
  region, so disjoint-byte writes under one group are safe).
* DVE/ACT alternate copying each 4-chunk PSUM bank to SBUF.
* Gate matmul, two modes:
  - mode="f32r": stationary w^T chunk [128h x 8e] (8-col LDWEIGHTS),
    moving x^T [128h x 256t] float32r (1 cyc/row at N>=256) accumulating
    logits^T [8 x 256]; then a transpose-mode flip back to [128t x 8e].
    float32r rounds operands to ~12 mantissa bits at PE ingest (HW
    measured: max rel 2.4e-4), which can flip a handful of near-tie
    top-2 decisions.
  - mode="f32x": bit-exact fp32.  x^T chunk is the *stationary* operand
    (LDWEIGHTS moves fp32 at 1 col / 1.2 GHz -- cheaper than streaming
    fp32 at 4 cyc/row) and w^T streams 8 columns, accumulating
    [128t x 8e] per tile directly (no flip-back needed).
* Gate matmuls for group g are interleaved among group g+1's transposes
  so the PE never waits on the PSUM->SBUF copies.
* Top-2 via DVE max/max_index; renormalized weights reduce to
  w1 = 1/(1+exp(m2-m1)), w2 = 1-w1 (the softmax denominator cancels).

Walrus's TPB instruction encodings carry a single sync-wait slot, so a
post-pass hoists surplus Tile-generated waits onto same-engine
EventSemaphore prefix instructions (semantics-preserving).
"""

import numpy as np

H = 4096            # hidden size
E = 8               # experts
P = 128             # SBUF partitions
C = H // P          # 32 128-wide h-chunks
T_TILE = 128        # tokens per tile
N_CORES = 8
TOKENS_TOTAL = 4 * 4096
TOKENS_PER_CORE = TOKENS_TOTAL // N_CORES   # 2048
N_TILES = TOKENS_PER_CORE // T_TILE         # 16
GB = 2              # token tiles per group (gate N = GB*128 >= 256)
CPB = 4             # transpose chunks assembled per PSUM bank
BANK_F = 512        # fp32 elems per PSUM bank per partition


def _emit_body(nc, mybir, pools, consts, x_flat, wq, iq, n_tiles, carry,
               mode="f32r"):
    natpool, xtgpool, ltpool, cpool, tpsum, gpsum, bpsum = pools
    w_nat, ident, ident_f = consts
    f32 = mybir.dt.float32
    f32r = mybir.dt.float32r
    dt_x = f32r if mode == "f32r" else f32
    n_groups = n_tiles // GB

    def emit_one_gate(prev):
        pxtg, pgps, ptiles = prev
        c = carry["gate_c"]
        if c >= C * (1 if mode == "f32r" else GB):
            return False
        if mode == "f32r":
            nc.tensor.matmul(
                pgps[0][:, 0 : GB * T_TILE],
                w_nat[:, c, :],
                pxtg[:, c],
                start=(c == 0),
                stop=(c == C - 1),
            )
        else:
            tg, cc = divmod(c, C)
            nc.tensor.matmul(
                pgps[tg][:, 0:E],
                pxtg[:, cc, tg, :],
                w_nat[:, cc, :],
                start=(cc == 0),
                stop=(cc == C - 1),
            )
        carry["gate_c"] = c + 1
        return True

    def finish_group(prev):
        """Drain remaining gates, then top-2 for the previous group."""
        pxtg, pgps, ptiles = prev
        while emit_one_gate(prev):
            pass
        if mode == "f32r":
            lt = ltpool.tile([E, GB * T_TILE], f32, tag="lt", name="lt")
            nc.scalar.copy(lt[:], pgps[0][:, 0 : GB * T_TILE])
            for tg, i in enumerate(ptiles):
                pb = bpsum.tile([P, BANK_F], f32, tag="bp", name="pb")
                # transpose-mode flip [8e x 128t] -> [128t x 8e]
                nc.tensor.matmul(
                    pb[:, 0:E],
                    lt[:, tg * T_TILE : (tg + 1) * T_TILE],
                    ident_f[0:E, 0:E],
                    start=True,
                    stop=True,
                    is_transpose=True,
                )
                nc.vector.tensor_copy(cpool.logits[:, i, :], pb[:, 0:E])
        else:
            for tg, i in enumerate(ptiles):
                nc.vector.tensor_copy(
                    cpool.logits[:, i, :], pgps[tg][:, 0:E]
                )
        for tg, i in enumerate(ptiles):
            nc.vector.max(out=cpool.sorted_w[:, i], in_=cpool.logits[:, i, :])
            nc.vector.max_index(
                out=cpool.idx_w[:, i], in_max=cpool.sorted_w[:, i],
                in_values=cpool.logits[:, i, :],
            )

    for g in range(n_groups):
        tiles = [g * GB + k for k in range(GB)]
        xts = []
        for i in tiles:
            xt = natpool.tile([P, H], dt_x, tag="xt", name="xt")
            nc.sync.dma_start(xt[:], x_flat[i * T_TILE : (i + 1) * T_TILE, :])
            xts.append(xt)
        xtg = xtgpool.tile([P, C, GB, T_TILE], dt_x, tag="xtg", name="xtg")
        if mode == "f32r":
            gp = gpsum.tile([E, BANK_F], f32, tag="gp", name="gp")
            gps = [gp]
        else:
            gps = [
                gpsum.tile([P, BANK_F], f32, tag="gp", name="gp")
                for _ in range(GB)
            ]
        prev = carry["prev"]
        asm = 0
        for tg, xt in enumerate(xts):
            for cb in range(C // CPB):
                pst = tpsum.tile([P, CPB, T_TILE], dt_x, tag="pst", name="pst")
                for k in range(CPB):
                    c = cb * CPB + k
                    nc.tensor.matmul(
                        pst[:, k],
                        xt[:, c * P : (c + 1) * P],
                        ident[:],
                        start=(k == 0),
                        stop=(k == CPB - 1),
                        is_transpose=True,
                    )
                dst = xtg[:, cb * CPB : (cb + 1) * CPB, tg, :]
                if asm % 2 == 0:
                    nc.vector.tensor_copy(dst, pst[:])
                else:
                    nc.scalar.copy(dst, pst[:])
                asm += 1
                if prev is not None:
                    emit_one_gate(prev)
                    emit_one_gate(prev)
                    if mode == "f32x":
                        emit_one_gate(prev)
                        emit_one_gate(prev)
        if prev is not None:
            finish_group(prev)
        carry["prev"] = (xtg, gps, tiles)
        carry["gate_c"] = 0

    # drain the final group of this rep
    finish_group(carry["prev"])
    carry["prev"] = None

    sorted_w, idx_w = cpool.sorted_w, cpool.idx_w
    # Batched renormalization over all tiles: w1 = 1/(1+e^(m2-m1)),
    # w2 = e^(m2-m1)/(1+e^(m2-m1)).
    m1 = sorted_w[:, :, 0]
    m2 = sorted_w[:, :, 1]
    d = cpool.tile([P, n_tiles], f32, tag="d", name="d")
    nc.vector.tensor_sub(d[:], m2, m1)
    t = cpool.tile([P, n_tiles], f32, tag="t", name="t")
    nc.scalar.activation(t[:], d[:], mybir.ActivationFunctionType.Exp)
    denom = cpool.tile([P, n_tiles], f32, tag="denom", name="denom")
    nc.vector.tensor_scalar_add(denom[:], t[:], 1.0)
    r = cpool.tile([P, n_tiles], f32, tag="r", name="r")
    nc.vector.reciprocal(r[:], denom[:])

    wout = cpool.tile([P, n_tiles, 2], f32, tag="wout", name="wout")
    nc.vector.tensor_copy(wout[:, :, 0], r[:])
    nc.vector.tensor_mul(wout[:, :, 1], t[:], r[:])
    iout = cpool.tile([P, n_tiles, 2], mybir.dt.uint32, tag="iout", name="iout")
    nc.vector.tensor_copy(iout[:], idx_w[:, :, 0:2])

    nc.gpsimd.dma_start(wq[:], wout[:])
    nc.gpsimd.dma_start(iq[:], iout[:])


def _legalize_sync_waits(nc, mybir):
    """Split surplus sync waits onto EventSemaphore prefix instructions.

    Walrus's TPB instruction structs have a single `events` wait slot and
    reject instructions with more sync waits.  The same engine sequencer
    executes an EventSemaphore (CTRL_ES) wait-only instruction in program
    order, so hoisting all but one wait onto ES prefixes is
    semantics-preserving.
    """
    limit = 1
    n = 0
    for bb in nc.main_func.blocks:
        out, changed = [], False
        for ins in bb.instructions:
            si = ins.sync_info
            if si is not None and len(si.on_wait) > limit:
                waits = list(si.on_wait)
                for w in waits[:-limit]:
                    es = mybir.InstEventSemaphore(
                        name=f"ESleg-{n}", engine=ins.engine, ins=[], outs=[],
                        sync_info=mybir.SyncInfo(on_wait=[w], on_update=[]),
                    )
                    out.append(es)
                    n += 1
                ins.sync_info = mybir.SyncInfo(
                    on_wait=waits[-limit:], on_update=list(si.on_update)
                )
                changed = True
            out.append(ins)
        if changed:
            bb.instructions = out
    return n


def build_program(tokens_per_core: int = TOKENS_PER_CORE, reps: int = 1,
                  legalize: bool = True, mode: str = "f32r",
                  nat_bufs: int = 4, tp_bufs: int = None):
    import concourse.bass as bass
    import concourse.mybir as mybir
    from concourse.masks import make_identity
    from concourse.tile import TileContext

    f32 = mybir.dt.float32
    f32r = mybir.dt.float32r
    dt_x = f32r if mode == "f32r" else f32
    # PSUM is 8 banks: transposes + gate accumulators + flip-back/w-build
    if tp_bufs is None:
        tp_bufs = 4 if mode == "f32r" else 3
    gp_bufs = 2 if mode == "f32r" else 2 * GB
    bp_bufs = 2 if mode == "f32r" else 1
    n_tiles = tokens_per_core // T_TILE
    nc = bass.Bass("TRN2", debug=False)
    x = nc.declare_dram_parameter(
        "x", [tokens_per_core, H], dt_x, isOutput=False
    )
    w = nc.declare_dram_parameter("w", [E, H], dt_x, isOutput=False)
    wq = nc.declare_dram_parameter(
        "wq", [P, n_tiles, 2], f32, isOutput=True
    )
    iq = nc.declare_dram_parameter(
        "iq", [P, n_tiles, 2], mybir.dt.uint32, isOutput=True
    )
    x_flat = x[:]

    with TileContext(nc) as tc:
        with (
            tc.tile_pool(name="cpool", bufs=1) as cpool,
            tc.tile_pool(name="natpool", bufs=nat_bufs) as natpool,
            tc.tile_pool(name="xtgpool", bufs=2) as xtgpool,
            tc.tile_pool(name="ltpool", bufs=2) as ltpool,
            tc.tile_pool(name="tpsum", bufs=tp_bufs, space="PSUM") as tpsum,
            tc.tile_pool(name="gpsum", bufs=gp_bufs, space="PSUM") as gpsum,
            tc.tile_pool(name="bpsum", bufs=bp_bufs, space="PSUM") as bpsum,
        ):
            ident_f = cpool.tile([P, P], f32, name="ident_f")
            make_identity(nc, ident_f[:])
            if mode == "f32r":
                ident = cpool.tile([P, P], f32r, name="ident")
                nc.sync.dma_start(ident[:], ident_f[:].bitcast(f32r))
            else:
                ident = ident_f

            # W^T with h%128 on partitions: w_nat[p, c, e] = W[e, c*128+p],
            # built with one-time PE transposes of the natural W rows.
            w_rows = cpool.tile([E, H], dt_x, name="w_rows")
            nc.sync.dma_start(w_rows[:], w[:])
            w_nat = cpool.tile([P, C, E], dt_x, name="w_nat")
            for c in range(C):
                psw = bpsum.tile([P, BANK_F], dt_x, tag="bp", name="psw")
                nc.tensor.matmul(
                    psw[:, 0:E], w_rows[:, c * P : (c + 1) * P],
                    ident[0:E, 0:E], start=True, stop=True, is_transpose=True,
                )
                nc.vector.tensor_copy(w_nat[:, c, :], psw[:, 0:E])

            cpool.logits = cpool.tile([P, n_tiles, E], f32, name="logits")
            cpool.sorted_w = cpool.tile([P, n_tiles, E], f32, name="sortw")
            cpool.idx_w = cpool.tile(
                [P, n_tiles, E], mybir.dt.uint32, name="idxw"
            )

            pools = (natpool, xtgpool, ltpool, cpool, tpsum, gpsum, bpsum)
            carry = {"prev": None, "gate_c": 0}
            for _rep in range(reps):
                _emit_body(
                    nc, mybir, pools, (w_nat, ident, ident_f), x_flat, wq, iq,
                    n_tiles, carry, mode=mode,
                )
    if legalize:
        _legalize_sync_waits(nc, mybir)
    return nc


def shard_inputs(hidden_states, gate_weight):
    hs = np.ascontiguousarray(np.asarray(hidden_states, dtype=np.float32)).reshape(
        TOKENS_TOTAL, H
    )
    gw = np.ascontiguousarray(np.asarray(gate_weight, dtype=np.float32))
    return [
        {"x": hs[c * TOKENS_PER_CORE : (c + 1) * TOKENS_PER_CORE], "w": gw}
        for c in range(N_CORES)
    ]


def assemble(results):
    ws, idxs = [], []
    for c in range(N_CORES):
        wq = np.asarray(results[c]["wq"]).reshape(P, N_TILES, 2)
        iq = np.asarray(results[c]["iq"]).reshape(P, N_TILES, 2)
        # token (core-local) = tile*128 + p
        ws.append(np.transpose(wq, (1, 0, 2)).reshape(TOKENS_PER_CORE, 2))
        idxs.append(np.transpose(iq, (1, 0, 2)).reshape(TOKENS_PER_CORE, 2))
    w_full = np.concatenate(ws, 0).reshape(4, 4096, 2).astype(np.float32)
    i_full = np.concatenate(idxs, 0).reshape(4, 4096, 2).astype(np.int32)
    return w_full, i_full


BEST_CONFIG = {"mode": "f32x"}


def kernel(hidden_states, gate_weight):
    from concourse.bass_utils import run_bass_kernel_spmd

    nc = build_program(**BEST_CONFIG)
    in_maps = shard_inputs(hidden_states, gate_weight)
    br = run_bass_kernel_spmd(nc, in_maps, list(range(N_CORES)), trace=False)
    return assemble(br.results)
